# revision 15
# baseline (speedup 1.0000x reference)
"""Multi-head attention kernel for Trainium2, 8 NeuronCores.

Problem: B=4, T=2048, D_in=1024, 16 heads x 64 dim, E=1024 (fp32).

Sharding: (batch x head-group). Core c handles batch b=c//2 and head-group
g=c%2 (8 heads, 512 qk/v dims). Each core computes its batch's QKV
projections restricted to its heads, full attention for those heads, and a
partial output projection. The host sums the two partial projections per
batch (the only cross-core reduction) and stacks batches.

Per-core dataflow (all matmuls bf16 inputs, fp32 PSUM accumulation):
  xT      = dma-xbar-transpose(cast_bf16(x))            [1024, 2048] per tensor
  qhT/khT = w.T @ xT   (weights stationary)             [512, 2048]
  vh_ext  = xT.T @ wv + ones column                     [2048, 8*65]
  S^T     = khT_h.T @ qhT_h per head pair               PSUM [128,1024]
  expS    = ACT exp(S^T/8) -> bf16 SBUF                 (the softmax exp)
  AVt     = es_block.T @ vh_ext  ("flipped": es is the stationary operand,
            the [128,65] v-slab is the moving operand)  PSUM [tq=128, 65]
            col 64 = softmax denominator (ones column)
  attn_n  = AVt[:, 0:64] * recip(AVt[:, 64])            per-partition scalar
  attnT   = dma-xbar-transpose(attn_n via DRAM)         [512, 2048] d-major
  y      += attnT_m.T @ wp_m  (K=128 contraction)       [2048, 1024] fp32

The emission order is produced by a small build-time list scheduler that
models per-engine clocks (PE matmul cost tracks the moving free size, ACT
exp ~1.04us per [128,1024] tile, DMA pipe ~360GB/s) so the in-order engine
queues stay busy: the scalar-engine exp stream is paced by the S matmuls
while QKV/projection chains fill the remaining PE slack.
"""

import sys

import numpy as np

if "/opt/trn_rl_repo" not in sys.path:
    sys.path.insert(0, "/opt/trn_rl_repo")

B, T, DIN = 4, 2048, 1024
NH, HD, EMB = 16, 64, 1024
HGD = 512          # per-core qk/v dims (8 heads * 64)
NKT = DIN // 128   # 8  input-dim k tiles
NQC = T // 512     # 4  t chunks of 512
NTT = T // 128     # 16 t tiles of 128
NM = HGD // 128    # 4  head-pair m tiles
HPC = 8            # heads per core
NW = 16            # windows (qc, pair)

_CACHE = {}

# clock model (ns): used only to choose emission order; correctness is
# semaphore-driven regardless of these estimates
MM = 1.0 / 2.4            # ns per moving column (bf16, warm PE)
EXP_NS = 1038.0           # ACT exp of a [128, 1024] tile
SEM = 130.0
ES_BUFS = 12
DVE_COPY = 660.0


def _build_nc():
    import concourse.bacc as bacc
    import concourse.mybir as mybir
    import concourse.tile as tile
    from contextlib import ExitStack

    dt = mybir.dt
    AF = mybir.ActivationFunctionType

    nc = bacc.Bacc("TRN2", target_bir_lowering=False, debug=False)
    xq = nc.declare_dram_parameter("xq", [T, DIN], dt.float32, isOutput=False)
    xk = nc.declare_dram_parameter("xk", [T, DIN], dt.float32, isOutput=False)
    xv = nc.declare_dram_parameter("xv", [T, DIN], dt.float32, isOutput=False)
    wq = nc.declare_dram_parameter("wq", [DIN, HGD], dt.float32, isOutput=False)
    wk = nc.declare_dram_parameter("wk", [DIN, HGD], dt.float32, isOutput=False)
    wv = nc.declare_dram_parameter("wv", [DIN, HGD], dt.float32, isOutput=False)
    wp = nc.declare_dram_parameter("wp", [HGD, EMB], dt.float32, isOutput=False)
    y = nc.declare_dram_parameter("y", [T, EMB], dt.float32, isOutput=True)

    with tile.TileContext(nc) as tc, ExitStack() as ctx:
        p_w = ctx.enter_context(tc.tile_pool(name="weights", bufs=1))
        p_xt = ctx.enter_context(tc.tile_pool(name="xt", bufs=4))
        p_qkh = ctx.enter_context(tc.tile_pool(name="qkh", bufs=1))
        p_vh = ctx.enter_context(tc.tile_pool(name="vh", bufs=1))
        p_exps = ctx.enter_context(tc.tile_pool(name="exps", bufs=ES_BUFS))
        p_attn = ctx.enter_context(tc.tile_pool(name="attn", bufs=1))
        p_an = ctx.enter_context(tc.tile_pool(name="attn_n", bufs=4))
        p_norm = ctx.enter_context(tc.tile_pool(name="norm", bufs=2))
        p_y = ctx.enter_context(tc.tile_pool(name="ysb", bufs=2))
        p_ps = ctx.enter_context(tc.tile_pool(name="psum_s", bufs=2, space="PSUM"))
        p_av = ctx.enter_context(tc.tile_pool(name="psum_av", bufs=1, space="PSUM"))
        p_big = ctx.enter_context(tc.tile_pool(name="psum_big", bufs=2, space="PSUM"))
        av_pers = None  # set below: persistent AV accumulators, one per head

        # bf16 copies of the inputs (DRAM->DRAM cast), transposed-read later
        xqb = nc.dram_tensor("xqb", [T, DIN], dt.bfloat16)
        xkb = nc.dram_tensor("xkb", [T, DIN], dt.bfloat16)
        xvb = nc.dram_tensor("xvb", [T, DIN], dt.bfloat16)
        # normalized attention, t-major, staged for the xbar transpose
        attn_d = nc.dram_tensor("attn_d", [T, HGD], dt.bfloat16)

        # --- persistent SBUF ---
        wq_sb = p_w.tile([128, NKT, HGD], dt.bfloat16, tag="wq")
        wk_sb = p_w.tile([128, NKT, HGD], dt.bfloat16, tag="wk")
        wv_sb = p_w.tile([128, NKT, HGD], dt.bfloat16, tag="wv")
        wp_sb = p_w.tile([128, NM, EMB], dt.bfloat16, tag="wp")

        qhT = [p_qkh.tile([128, T], dt.bfloat16, tag=f"qhT{m}", name=f"qhT{m}") for m in range(NM)]
        khT = [p_qkh.tile([128, T], dt.bfloat16, tag=f"khT{m}", name=f"khT{m}") for m in range(NM)]
        vh_ext = [p_vh.tile([128, HPC, HD + 1], dt.bfloat16, tag=f"vh{t_}", name=f"vh{t_}") for t_ in range(NTT)]
        for t_ in range(NTT):
            nc.vector.memset(vh_ext[t_][:, :, HD : HD + 1], 1.0)
        attnT = [p_attn.tile([128, T], dt.bfloat16, tag=f"at{m}", name=f"at{m}") for m in range(NM)]
        av_pers = (
            p_av.tile([128, 4, HD + 1], dt.float32, tag="ava", name="ava"),
            p_av.tile([128, 4, HD + 1], dt.float32, tag="avb", name="avb"),
        )

        # ================= staging: casts (SWDGE) + xposes (SP) ============
        pipe = [0.0]
        xts = {}    # (tensor, block, half) -> (xt tile, col offset, est)
        n_load_T = [0]
        srcs = {"q": (xqb, xq), "k": (xkb, xk), "v": (xvb, xv)}

        def cast(tname, lo, hi):
            xb_d, xs = srcs[tname]
            nc.gpsimd.dma_start(out=xb_d[lo:hi, :], in_=xs[lo:hi, :])
            pipe[0] += (hi - lo) * DIN * 2 / 360.0 + 200

        def wload(dst, src, pat):
            nc.gpsimd.dma_start(out=dst[:], in_=src.rearrange(pat, p=128))
            pipe[0] += 2912 + 200
            return pipe[0] + 1500

        # Casts are emitted eagerly (DRAM scratch, no WAR hazard). XPOSEs are
        # deferred: the xt pool has 4 slots, so an XPOSE emitted too early
        # would carry a WAR against reader chains that appear later in
        # program order (deadlock). ensure_staged() emits each XPOSE on
        # first demand, after force-emitting the remaining readers of the
        # slot being evicted.
        stagers = {}
        loads_emitted = []
        readers = {}

        def plan(tname, b, half=None):
            lo = 512 * b + (256 * half if half is not None else 0)
            w = 256 if half is not None else 512
            cast(tname, lo, lo + w)
            pipe[0] += (w // 16) * (DIN // 128) * 14 + 200
            stagers[(tname, b, half)] = {"est": pipe[0] + 1500, "done": False}

        west = {}
        west["wq"] = wload(wq_sb, wq, "(kt p) n -> p kt n")
        plan("q", 0, 0)
        plan("q", 0, 1)
        west["wk"] = wload(wk_sb, wk, "(kt p) n -> p kt n")
        plan("k", 0, 0)
        west["wv"] = wload(wv_sb, wv, "(kt p) n -> p kt n")
        plan("v", 0, 0)
        plan("k", 0, 1)
        plan("v", 0, 1)
        plan("k", 1)
        plan("v", 1)
        plan("q", 1)
        west["wp"] = wload(wp_sb, wp, "(m p) e -> p m e")
        plan("k", 2)
        plan("v", 2)
        plan("q", 2)
        plan("k", 3)
        plan("v", 3)
        plan("q", 3)

        def do_load(key):
            # XPOSE has a single semaphore-wait slot; reused pool slots would
            # need WAR+RAW, so a tiny DMA first touches the source chunk and
            # the destination tile, absorbing both waits.
            tname, b, half = key
            xb_d, _ = srcs[tname]
            lo = 512 * b + (256 * half if half is not None else 0)
            w = 256 if half is not None else 512
            xt = p_xt.tile([128, NKT, w], dt.bfloat16, tag="xt", name="xt")
            if n_load_T[0] >= 4:
                row = xb_d[lo : lo + 1, 0:NKT]
                nc.sync.dma_start(out=xt[:, :, 0:1], in_=row.to_broadcast([128, NKT]))
            n_load_T[0] += 1
            nc.sync.dma_start(out=xt[:], in_=xb_d[lo : lo + w, :], transpose=True)
            est = stagers[key]["est"]
            if half is None:
                xts[(tname, b, 0)] = (xt, 0, est)
                xts[(tname, b, 1)] = (xt, 256, est)
            else:
                xts[(tname, b, half)] = (xt, 0, est)

        def ensure_staged(key):
            st = stagers[key]
            if st["done"]:
                return
            n = len(loads_emitted)
            if n >= 4:
                for cid in readers.get(loads_emitted[n - 4], []):
                    run_chain(cid)
            st["done"] = True
            loads_emitted.append(key)
            do_load(key)

        # ================= QKV projection chains (PE fillers) ==============
        # chain id -> dict(ready est, dur ns, emit fn). Block-0 chains come
        # in 256-column halves (matching the staged halves).
        chains = {}
        chain_order = []

        def add_chain(cid, key, ready, dur, fn):
            chains[cid] = {"ready": ready, "dur": dur, "fn": fn, "done": False,
                           "key": key}
            chain_order.append(cid)
            readers.setdefault(key, []).append(cid)

        def emit_pqk(dst, wsb, tname, b, half, m):
            xt, co, _ = xts[(tname, b, half)]
            ncol = 256 if b == 0 else 512
            # uniform [128, 512] tiles: slot keys include the byte size, so
            # a second size class would cost two extra PSUM banks
            ps = p_big.tile([128, 512], dt.float32, tag="psb", name="psb")
            for kt in range(NKT):
                nc.tensor.matmul(
                    ps[:, 0:ncol], wsb[:, kt, 128 * m : 128 * (m + 1)],
                    xt[:, kt, co : co + ncol] if b != 0 else xt[:, kt, 0:ncol],
                    start=(kt == 0), stop=(kt == NKT - 1),
                )
            lo = 512 * b + (256 * half if b == 0 else 0)
            nc.vector.tensor_copy(dst[m][:, lo : lo + ncol], ps[:, 0:ncol])

        def emit_pv(b, half, ti):
            # ti indexes the 128-token tile within the block
            xt, co, _ = xts[("v", b, half if b == 0 else ti // 2)]
            tt = 4 * b + ti
            off = (co + 128 * (ti % 2)) if b != 0 else 128 * (ti % 2)
            ps = p_big.tile([128, HGD], dt.float32, tag="psb", name="psb")
            for kt in range(NKT):
                nc.tensor.matmul(
                    ps[:], xt[:, kt, off : off + 128], wv_sb[:, kt, :],
                    start=(kt == 0), stop=(kt == NKT - 1),
                )
            nc.vector.tensor_copy(
                vh_ext[tt][:, :, 0:HD], ps.rearrange("p (h d) -> p h d", h=HPC)
            )

        for b in range(NQC):
            halves = (0, 1) if b == 0 else (0,)
            for m in range(NM):
                for h in halves:
                    kk = ("k", b, h if b == 0 else None)
                    qk = ("q", b, h if b == 0 else None)
                    dur = 8 * (256 if b == 0 else 512) * MM
                    add_chain(("pk", b, m, h), kk,
                              max(stagers[kk]["est"], west["wk"]), dur,
                              lambda b=b, m=m, h=h: emit_pqk(khT, wk_sb, "k", b, h, m))
                    add_chain(("pq", b, m, h), qk,
                              max(stagers[qk]["est"], west["wq"]), dur,
                              lambda b=b, m=m, h=h: emit_pqk(qhT, wq_sb, "q", b, h, m))
            for ti in range(4):
                h = ti // 2
                vk = ("v", b, h if b == 0 else None)
                add_chain(("pv", b, ti), vk,
                          max(stagers[vk]["est"], west["wv"]), 8 * HGD * MM,
                          lambda b=b, ti=ti, h=h: emit_pv(b, h, ti))

        # ================= attention unit emitters =========================
        windows = [(qc, pair) for qc in range(NQC) for pair in range(NM)]
        av_tiles = {}
        es_tiles = {}

        def emit_S(w, kt):
            qc, pair = windows[w]
            qsl = slice(512 * qc, 512 * (qc + 1))
            ksl = slice(128 * kt, 128 * (kt + 1))
            ps = p_ps.tile([128, 1024], dt.float32, tag="pss", name="pss")
            nc.tensor.matmul(ps[:, 0:512], khT[pair][0:64, ksl], qhT[pair][0:64, qsl],
                             start=True, stop=True)
            nc.tensor.matmul(ps[:, 512:1024], khT[pair][64:128, ksl], qhT[pair][64:128, qsl],
                             start=True, stop=True)
            es = p_exps.tile([128, 1024], dt.bfloat16, tag="es", name="es")
            nc.scalar.activation(es[:], ps[:], AF.Exp, scale=1.0 / 8.0)
            es_tiles[(w, kt)] = es

        def emit_AV(w, kt):
            qc, pair = windows[w]
            if kt == 0:
                av_tiles[w] = av_pers
            es = es_tiles.pop((w, kt))
            for h in range(2):
                av = av_tiles[w][h]
                for tqb in range(4):
                    # every av matmul closes its own group (stop is sim-side
                    # bookkeeping only) so no accumulation group stays open
                    # across the interleaved S/chain matmuls
                    nc.tensor.matmul(
                        av[:, tqb, :],
                        es[:, 512 * h + 128 * tqb : 512 * h + 128 * (tqb + 1)],
                        vh_ext[kt][:, 2 * pair + h, :],
                        start=(kt == 0 and tqb == 0),
                        stop=True,
                        skip_group_check=True,
                    )

        def emit_norm(w):
            qc, pair = windows[w]
            av_a, av_b = av_tiles.pop(w)
            sts, rcs = [], []
            for h, av in ((0, av_a), (1, av_b)):
                st = p_norm.tile([128, 4, HD + 1], dt.float32, tag=f"st{h}", name=f"st{h}")
                nc.vector.tensor_copy(st[:], av[:])
                rc = p_norm.tile([128, 4, 1], dt.float32, tag=f"rc{h}", name=f"rc{h}")
                nc.vector.reciprocal(rc[:], st[:, :, HD : HD + 1])
                sts.append(st)
                rcs.append(rc)
            an = p_an.tile([128, 4, 2, HD], dt.bfloat16, tag="an", name="an")
            for tqb in range(4):
                for h in range(2):
                    nc.vector.tensor_scalar_mul(
                        an[:, tqb, h, :], sts[h][:, tqb, 0:HD], rcs[h][:, tqb, 0:1]
                    )
            qsl = slice(512 * qc, 512 * (qc + 1))
            csl = slice(128 * pair, 128 * (pair + 1))
            nc.sync.dma_start(
                out=attn_d[qsl, csl].rearrange("(tb p) (h d) -> p tb h d", p=128, h=2),
                in_=an[:],
            )
            nc.sync.dma_start(out=attnT[pair][:, qsl], in_=attn_d[qsl, csl], transpose=True)

        def emit_proj(qc, tt, ec):
            tsl = slice(128 * tt, 128 * (tt + 1))
            esl = slice(512 * ec, 512 * (ec + 1))
            ps = p_big.tile([128, 512], dt.float32, tag="psb", name="psb")
            for m in range(NM):
                nc.tensor.matmul(ps[:], attnT[m][:, tsl], wp_sb[:, m, esl],
                                 start=(m == 0), stop=(m == NM - 1))
            ysb = p_y.tile([128, 512], dt.float32, tag="ysb", name="ysb")
            nc.vector.tensor_copy(ysb[:], ps[:])
            nc.gpsimd.dma_start(out=y[tsl, esl], in_=ysb[:])

        # ================= build-time list scheduler =======================
        # Global S order honours staging epochs: epoch(w, kt) =
        # max(qc, kt//4); within an epoch, window-major so windows finish
        # (and release norm + projection work) as early as possible.
        s_order = sorted(
            [(w, kt) for w in range(NW) for kt in range(NTT)],
            key=lambda u: (max(u[0] // NM, u[1] // 4), u[0], u[1]),
        )

        def s_deps(w, kt):
            qc, pair = windows[w]
            b, half = kt // 4, (kt % 4) // 2
            d = []
            if qc == 0:
                d += [("pq", 0, pair, 0), ("pq", 0, pair, 1)]
            else:
                d += [("pq", qc, pair, 0)]
            if b == 0:
                d += [("pk", 0, pair, h) for h in range(half + 1)]
            else:
                d += [("pk", b, pair, 0)]
            return d

        def av_deps(w, kt):
            return [("pv", kt // 4, kt % 4)]

        t_pe = [0.0]
        t_act = [0.0]
        t_dve = [0.0]
        exp_end = {}

        def run_chain(cid):
            c = chains[cid]
            if c["done"]:
                return
            c["done"] = True
            ensure_staged(c["key"])
            t_pe[0] = max(t_pe[0], c["ready"]) + c["dur"]
            t_dve[0] = max(t_dve[0], t_pe[0] + SEM) + DVE_COPY
            c["fn"]()

        def s_ready_est(idx, w, kt):
            r = 0.0
            for cid in s_deps(w, kt):
                c = chains[cid]
                r = max(r, (c["ready"] + c["dur"] if not c["done"] else 0.0))
            if idx >= 2:
                r = max(r, exp_end[idx - 2] + SEM)
            return r

        proj_q = []          # (ready est, qc, tt, ec)
        xb_est = {}          # (qc, pair) -> est attnT ready
        s_idx = [0]
        av_idx = [0]

        def do_S():
            w, kt = s_order[s_idx[0]]
            for cid in s_deps(w, kt):
                run_chain(cid)
            t_pe[0] = max(t_pe[0], s_ready_est(s_idx[0], w, kt)) + 2 * 512 * MM
            emit_S(w, kt)
            t_act[0] = max(t_act[0], t_pe[0] + SEM) + EXP_NS
            exp_end[s_idx[0]] = t_act[0]
            s_idx[0] += 1

        def do_AV():
            w, kt = s_order[av_idx[0]]
            for cid in av_deps(w, kt):
                run_chain(cid)
            t_pe[0] = max(t_pe[0], exp_end[av_idx[0]] + SEM) + 8 * 65 * MM
            emit_AV(w, kt)
            av_idx[0] += 1
            if kt == NTT - 1:
                qc, pair = windows[w]
                emit_norm(w)
                t_dve[0] = max(t_dve[0], t_pe[0] + SEM) + 2300
                xb_est[(qc, pair)] = t_dve[0] + 6000
                if pair == NM - 1:
                    rdy = max(xb_est[(qc, p)] for p in range(NM))
                    for tt in range(4 * qc, 4 * qc + 4):
                        for ec in range(2):
                            proj_q.append([rdy, qc, tt, ec])

        def next_filler():
            # first unemitted QKV chain that is ready, else a ready proj tile
            for cid in chain_order:
                c = chains[cid]
                if not c["done"] and c["ready"] <= t_pe[0] + 100:
                    return ("chain", cid)
            for it in proj_q:
                if it[0] <= t_pe[0] + 100:
                    return ("proj", it)
            return None

        def do_filler(f):
            kind, it = f
            if kind == "chain":
                run_chain(it)
            else:
                proj_q.remove(it)
                t_pe[0] = max(t_pe[0], it[0]) + 4 * 512 * MM
                emit_proj(it[1], it[2], it[3])
                t_dve[0] = max(t_dve[0], t_pe[0] + SEM) + DVE_COPY

        NSU = len(s_order)
        while s_idx[0] < NSU or av_idx[0] < NSU or proj_q or not all(
            c["done"] for c in chains.values()
        ):
            can_S = s_idx[0] < NSU
            can_AV = av_idx[0] < min(s_idx[0], NSU)
            s_stall = (
                max(0.0, s_ready_est(s_idx[0], *s_order[s_idx[0]]) - t_pe[0])
                if can_S else float("inf")
            )
            av_stall = (
                max(0.0, exp_end[av_idx[0]] + SEM - t_pe[0]) if can_AV else float("inf")
            )
            if can_AV and s_idx[0] - av_idx[0] >= ES_BUFS - 4:
                do_AV()
                continue
            if can_S and s_stall <= 30:
                do_S()
                continue
            if can_AV and av_stall <= 30:
                do_AV()
                continue
            f = next_filler()
            if f is not None:
                do_filler(f)
                continue
            if can_AV and av_stall <= s_stall:
                do_AV()
            elif can_S:
                do_S()
            elif can_AV:
                do_AV()
            elif proj_q:
                it = min(proj_q, key=lambda x: x[0])
                do_filler(("proj", it))
            else:
                # stragglers: emit remaining chains by readiness
                rem = [cid for cid in chain_order if not chains[cid]["done"]]
                run_chain(min(rem, key=lambda c: chains[c]["ready"]))

    nc.compile()
    return nc


def _get_nc():
    if "nc" not in _CACHE:
        _CACHE["nc"] = _build_nc()
    return _CACHE["nc"]


def core_input_map(k, q, v, w_key, w_query, w_value, w_proj, core):
    b, g = core // 2, core % 2
    sl = slice(g * HGD, (g + 1) * HGD)
    f32 = np.float32
    return {
        "xq": np.ascontiguousarray(q[b], dtype=f32),
        "xk": np.ascontiguousarray(k[b], dtype=f32),
        "xv": np.ascontiguousarray(v[b], dtype=f32),
        "wq": np.ascontiguousarray(w_query[:, sl], dtype=f32),
        "wk": np.ascontiguousarray(w_key[:, sl], dtype=f32),
        "wv": np.ascontiguousarray(w_value[:, sl], dtype=f32),
        "wp": np.ascontiguousarray(w_proj[sl, :], dtype=f32),
    }


def kernel(k, q, v, w_key, w_query, w_value, w_proj):
    from concourse.bass_utils import run_bass_kernel_spmd

    nc = _get_nc()
    in_maps = [
        core_input_map(k, q, v, w_key, w_query, w_value, w_proj, c) for c in range(8)
    ]
    res = run_bass_kernel_spmd(nc, in_maps, list(range(8))).results
    out = np.empty((B, T, EMB), np.float32)
    for b in range(B):
        out[b] = res[2 * b]["y"] + res[2 * b + 1]["y"]
    return out


# revision 19
# speedup vs baseline: 1.0247x; 1.0247x over previous
"""Multi-head attention kernel for Trainium2, 8 NeuronCores.

Problem: B=4, T=2048, D_in=1024, 16 heads x 64 dim, E=1024 (fp32).

Sharding: (batch x head-group). Core c handles batch b=c//2 and head-group
g=c%2 (8 heads, 512 qk/v dims). Each core computes its batch's QKV
projections restricted to its heads, full attention for those heads, and a
partial output projection. The host sums the two partial projections per
batch (the only cross-core reduction) and stacks batches.

Per-core dataflow (all matmuls bf16 inputs, fp32 PSUM accumulation):
  xT      = dma-xbar-transpose(cast_bf16(x))            [1024, 2048] per tensor
  qhT/khT = w.T @ xT   (weights stationary)             [512, 2048]
  vh_ext  = xT.T @ wv + ones column                     [2048, 8*65]
  S^T     = khT_h.T @ qhT_h per head pair               PSUM [128,1024]
  expS    = ACT exp(S^T/8) -> bf16 SBUF                 (the softmax exp)
  AVt     = es_block.T @ vh_ext  ("flipped": es is the stationary operand,
            the [128,65] v-slab is the moving operand)  PSUM [tq=128, 65]
            col 64 = softmax denominator (ones column)
  attn_n  = AVt[:, 0:64] * recip(AVt[:, 64])            per-partition scalar
  attnT   = dma-xbar-transpose(attn_n via DRAM)         [512, 2048] d-major
  y      += attnT_m.T @ wp_m  (K=128 contraction)       [2048, 1024] fp32

The emission order is produced by a small build-time list scheduler that
models per-engine clocks (PE matmul cost tracks the moving free size, ACT
exp ~1.04us per [128,1024] tile, DMA pipe ~360GB/s) so the in-order engine
queues stay busy: the scalar-engine exp stream is paced by the S matmuls
while QKV/projection chains fill the remaining PE slack.
"""

import sys

import numpy as np

if "/opt/trn_rl_repo" not in sys.path:
    sys.path.insert(0, "/opt/trn_rl_repo")

B, T, DIN = 4, 2048, 1024
NH, HD, EMB = 16, 64, 1024
HGD = 512          # per-core qk/v dims (8 heads * 64)
NKT = DIN // 128   # 8  input-dim k tiles
NQC = T // 512     # 4  t chunks of 512
NTT = T // 128     # 16 t tiles of 128
NM = HGD // 128    # 4  head-pair m tiles
HPC = 8            # heads per core
NW = 16            # windows (qc, pair)

_CACHE = {}

# clock model (ns): used only to choose emission order; correctness is
# semaphore-driven regardless of these estimates
MM = 1.0 / 2.4            # ns per moving column (bf16, warm PE)
EXP_NS = 1038.0           # ACT exp of a [128, 1024] tile
SEM = 130.0
ES_BUFS = 12
DVE_COPY = 660.0


def _build_nc():
    import concourse.bacc as bacc
    import concourse.mybir as mybir
    import concourse.tile as tile
    from contextlib import ExitStack

    dt = mybir.dt
    AF = mybir.ActivationFunctionType

    nc = bacc.Bacc("TRN2", target_bir_lowering=False, debug=False)
    xq = nc.declare_dram_parameter("xq", [T, DIN], dt.float32, isOutput=False)
    xk = nc.declare_dram_parameter("xk", [T, DIN], dt.float32, isOutput=False)
    xv = nc.declare_dram_parameter("xv", [T, DIN], dt.float32, isOutput=False)
    wq = nc.declare_dram_parameter("wq", [DIN, HGD], dt.float32, isOutput=False)
    wk = nc.declare_dram_parameter("wk", [DIN, HGD], dt.float32, isOutput=False)
    wv = nc.declare_dram_parameter("wv", [DIN, HGD], dt.float32, isOutput=False)
    wp = nc.declare_dram_parameter("wp", [HGD, EMB], dt.float32, isOutput=False)
    y = nc.declare_dram_parameter("y", [T, EMB], dt.float32, isOutput=True)

    with tile.TileContext(nc) as tc, ExitStack() as ctx:
        p_w = ctx.enter_context(tc.tile_pool(name="weights", bufs=1))
        p_xt = ctx.enter_context(tc.tile_pool(name="xt", bufs=4))
        p_qkh = ctx.enter_context(tc.tile_pool(name="qkh", bufs=1))
        p_vh = ctx.enter_context(tc.tile_pool(name="vh", bufs=1))
        p_exps = ctx.enter_context(tc.tile_pool(name="exps", bufs=ES_BUFS))
        p_attn = ctx.enter_context(tc.tile_pool(name="attn", bufs=1))
        p_an = ctx.enter_context(tc.tile_pool(name="attn_n", bufs=4))
        p_norm = ctx.enter_context(tc.tile_pool(name="norm", bufs=2))
        p_y = ctx.enter_context(tc.tile_pool(name="ysb", bufs=2))
        p_ps = ctx.enter_context(tc.tile_pool(name="psum_s", bufs=2, space="PSUM"))
        p_av = ctx.enter_context(tc.tile_pool(name="psum_av", bufs=1, space="PSUM"))
        p_big = ctx.enter_context(tc.tile_pool(name="psum_big", bufs=2, space="PSUM"))
        av_pers = None  # set below: persistent AV accumulators, one per head

        # bf16 copies of the inputs (DRAM->DRAM cast), transposed-read later
        xqb = nc.dram_tensor("xqb", [T, DIN], dt.bfloat16)
        xkb = nc.dram_tensor("xkb", [T, DIN], dt.bfloat16)
        xvb = nc.dram_tensor("xvb", [T, DIN], dt.bfloat16)
        # normalized attention, t-major, staged for the xbar transpose
        attn_d = nc.dram_tensor("attn_d", [T, HGD], dt.bfloat16)

        # --- persistent SBUF ---
        wq_sb = p_w.tile([128, NKT, HGD], dt.bfloat16, tag="wq")
        wk_sb = p_w.tile([128, NKT, HGD], dt.bfloat16, tag="wk")
        wv_sb = p_w.tile([128, NKT, HGD], dt.bfloat16, tag="wv")
        wp_sb = p_w.tile([128, NM, EMB], dt.bfloat16, tag="wp")

        qhT = [p_qkh.tile([128, T], dt.bfloat16, tag=f"qhT{m}", name=f"qhT{m}") for m in range(NM)]
        khT = [p_qkh.tile([128, T], dt.bfloat16, tag=f"khT{m}", name=f"khT{m}") for m in range(NM)]
        vh_ext = [p_vh.tile([128, HPC, HD + 1], dt.bfloat16, tag=f"vh{t_}", name=f"vh{t_}") for t_ in range(NTT)]
        for t_ in range(NTT):
            nc.vector.memset(vh_ext[t_][:, :, HD : HD + 1], 1.0)
        attnT = [p_attn.tile([128, T], dt.bfloat16, tag=f"at{m}", name=f"at{m}") for m in range(NM)]
        av_pers = (
            p_av.tile([128, 4, HD + 1], dt.float32, tag="ava", name="ava"),
            p_av.tile([128, 4, HD + 1], dt.float32, tag="avb", name="avb"),
        )

        # ================= staging: casts (SWDGE) + xposes (SP) ============
        pipe = [0.0]
        xts = {}    # (tensor, block, half) -> (xt tile, col offset, est)
        n_load_T = [0]
        srcs = {"q": (xqb, xq), "k": (xkb, xk), "v": (xvb, xv)}

        def cast(tname, lo, hi):
            xb_d, xs = srcs[tname]
            nc.gpsimd.dma_start(out=xb_d[lo:hi, :], in_=xs[lo:hi, :])
            pipe[0] += (hi - lo) * DIN * 2 / 360.0 + 200

        def wload(dst, src, pat):
            nc.gpsimd.dma_start(out=dst[:], in_=src.rearrange(pat, p=128))
            pipe[0] += 2912 + 200
            return pipe[0] + 1500

        # Casts are emitted eagerly (DRAM scratch, no WAR hazard). XPOSEs are
        # deferred: the xt pool has 4 slots, so an XPOSE emitted too early
        # would carry a WAR against reader chains that appear later in
        # program order (deadlock). ensure_staged() emits each XPOSE on
        # first demand, after force-emitting the remaining readers of the
        # slot being evicted.
        stagers = {}
        loads_emitted = []
        readers = {}

        def plan(tname, b, half=None):
            lo = 512 * b + (256 * half if half is not None else 0)
            w = 256 if half is not None else 512
            cast(tname, lo, lo + w)
            pipe[0] += (w // 16) * (DIN // 128) * 14 + 200
            stagers[(tname, b, half)] = {"est": pipe[0] + 1500, "done": False}

        west = {}
        west["wq"] = wload(wq_sb, wq, "(kt p) n -> p kt n")
        plan("q", 0, 0)
        plan("q", 0, 1)
        west["wk"] = wload(wk_sb, wk, "(kt p) n -> p kt n")
        plan("k", 0, 0)
        west["wv"] = wload(wv_sb, wv, "(kt p) n -> p kt n")
        plan("v", 0, 0)
        plan("k", 0, 1)
        plan("v", 0, 1)
        plan("k", 1)
        plan("v", 1)
        plan("q", 1)
        west["wp"] = wload(wp_sb, wp, "(m p) e -> p m e")
        plan("k", 2)
        plan("v", 2)
        plan("q", 2)
        plan("k", 3)
        plan("v", 3)
        plan("q", 3)

        def do_load(key):
            # XPOSE has a single semaphore-wait slot; reused pool slots would
            # need WAR+RAW, so a tiny DMA first touches the source chunk and
            # the destination tile, absorbing both waits.
            tname, b, half = key
            xb_d, _ = srcs[tname]
            lo = 512 * b + (256 * half if half is not None else 0)
            w = 256 if half is not None else 512
            xt = p_xt.tile([128, NKT, w], dt.bfloat16, tag="xt", name="xt")
            if n_load_T[0] >= 4:
                row = xb_d[lo : lo + 1, 0:NKT]
                nc.sync.dma_start(out=xt[:, :, 0:1], in_=row.to_broadcast([128, NKT]))
            n_load_T[0] += 1
            nc.sync.dma_start(out=xt[:], in_=xb_d[lo : lo + w, :], transpose=True)
            est = stagers[key]["est"]
            if half is None:
                xts[(tname, b, 0)] = (xt, 0, est)
                xts[(tname, b, 1)] = (xt, 256, est)
            else:
                xts[(tname, b, half)] = (xt, 0, est)

        def ensure_staged(key):
            st = stagers[key]
            if st["done"]:
                return
            tgt = evict_target(key)
            if tgt is not None:
                for cid in list(readers.get(tgt, [])):
                    run_chain(cid)
            st["done"] = True
            loads_emitted.append(key)
            do_load(key)

        # ================= QKV projection chains (PE fillers) ==============
        # chain id -> dict(ready est, dur ns, emit fn). Block-0 chains come
        # in 256-column halves (matching the staged halves).
        chains = {}
        chain_order = []

        def add_chain(cid, key, ready, dur, fn):
            chains[cid] = {"ready": ready, "dur": dur, "fn": fn, "done": False,
                           "key": key}
            chain_order.append(cid)
            readers.setdefault(key, []).append(cid)

        def emit_pqk(dst, wsb, tname, b, half, m):
            xt, co, _ = xts[(tname, b, half)]
            ncol = 256 if b == 0 else 512
            # uniform [128, 512] tiles: slot keys include the byte size, so
            # a second size class would cost two extra PSUM banks
            ps = p_big.tile([128, 512], dt.float32, tag="psb", name="psb")
            for kt in range(NKT):
                nc.tensor.matmul(
                    ps[:, 0:ncol], wsb[:, kt, 128 * m : 128 * (m + 1)],
                    xt[:, kt, co : co + ncol] if b != 0 else xt[:, kt, 0:ncol],
                    start=(kt == 0), stop=(kt == NKT - 1),
                )
            lo = 512 * b + (256 * half if b == 0 else 0)
            nc.vector.tensor_copy(dst[m][:, lo : lo + ncol], ps[:, 0:ncol])

        def emit_pv(b, half, ti):
            # ti indexes the 128-token tile within the block
            xt, co, _ = xts[("v", b, half if b == 0 else ti // 2)]
            tt = 4 * b + ti
            off = (co + 128 * (ti % 2)) if b != 0 else 128 * (ti % 2)
            ps = p_big.tile([128, HGD], dt.float32, tag="psb", name="psb")
            for kt in range(NKT):
                nc.tensor.matmul(
                    ps[:], xt[:, kt, off : off + 128], wv_sb[:, kt, :],
                    start=(kt == 0), stop=(kt == NKT - 1),
                )
            nc.vector.tensor_copy(
                vh_ext[tt][:, :, 0:HD], ps.rearrange("p (h d) -> p h d", h=HPC)
            )

        for b in range(NQC):
            halves = (0, 1) if b == 0 else (0,)
            for m in range(NM):
                for h in halves:
                    kk = ("k", b, h if b == 0 else None)
                    qk = ("q", b, h if b == 0 else None)
                    dur = 8 * (256 if b == 0 else 512) * MM
                    add_chain(("pk", b, m, h), kk,
                              max(stagers[kk]["est"], west["wk"]), dur,
                              lambda b=b, m=m, h=h: emit_pqk(khT, wk_sb, "k", b, h, m))
                    add_chain(("pq", b, m, h), qk,
                              max(stagers[qk]["est"], west["wq"]), dur,
                              lambda b=b, m=m, h=h: emit_pqk(qhT, wq_sb, "q", b, h, m))
            for ti in range(4):
                h = ti // 2
                vk = ("v", b, h if b == 0 else None)
                add_chain(("pv", b, ti), vk,
                          max(stagers[vk]["est"], west["wv"]), 8 * HGD * MM,
                          lambda b=b, ti=ti, h=h: emit_pv(b, h, ti))

        # ================= attention unit emitters =========================
        windows = [(qc, pair) for qc in range(NQC) for pair in range(NM)]
        av_tiles = {}
        es_tiles = {}

        def emit_S(w, kt):
            qc, pair = windows[w]
            qsl = slice(512 * qc, 512 * (qc + 1))
            ksl = slice(128 * kt, 128 * (kt + 1))
            ps = p_ps.tile([128, 1024], dt.float32, tag="pss", name="pss")
            nc.tensor.matmul(ps[:, 0:512], khT[pair][0:64, ksl], qhT[pair][0:64, qsl],
                             start=True, stop=True)
            nc.tensor.matmul(ps[:, 512:1024], khT[pair][64:128, ksl], qhT[pair][64:128, qsl],
                             start=True, stop=True)
            es = p_exps.tile([128, 1024], dt.bfloat16, tag="es", name="es")
            nc.scalar.activation(es[:], ps[:], AF.Exp, scale=1.0 / 8.0)
            es_tiles[(w, kt)] = es

        def emit_AV(w, kt):
            qc, pair = windows[w]
            if kt == 0:
                av_tiles[w] = av_pers
            es = es_tiles.pop((w, kt))
            for h in range(2):
                av = av_tiles[w][h]
                for tqb in range(4):
                    # every av matmul closes its own group (stop is sim-side
                    # bookkeeping only) so no accumulation group stays open
                    # across the interleaved S/chain matmuls
                    nc.tensor.matmul(
                        av[:, tqb, :],
                        es[:, 512 * h + 128 * tqb : 512 * h + 128 * (tqb + 1)],
                        vh_ext[kt][:, 2 * pair + h, :],
                        start=(kt == 0 and tqb == 0),
                        stop=True,
                        skip_group_check=True,
                    )

        def emit_norm(w):
            qc, pair = windows[w]
            av_a, av_b = av_tiles.pop(w)
            sts, rcs = [], []
            for h, av in ((0, av_a), (1, av_b)):
                st = p_norm.tile([128, 4, HD + 1], dt.float32, tag=f"st{h}", name=f"st{h}")
                nc.vector.tensor_copy(st[:], av[:])
                rc = p_norm.tile([128, 4, 1], dt.float32, tag=f"rc{h}", name=f"rc{h}")
                nc.vector.reciprocal(rc[:], st[:, :, HD : HD + 1])
                sts.append(st)
                rcs.append(rc)
            an = p_an.tile([128, 4, 2, HD], dt.bfloat16, tag="an", name="an")
            for tqb in range(4):
                for h in range(2):
                    nc.vector.tensor_scalar_mul(
                        an[:, tqb, h, :], sts[h][:, tqb, 0:HD], rcs[h][:, tqb, 0:1]
                    )
            qsl = slice(512 * qc, 512 * (qc + 1))
            csl = slice(128 * pair, 128 * (pair + 1))
            nc.sync.dma_start(
                out=attn_d[qsl, csl].rearrange("(tb p) (h d) -> p tb h d", p=128, h=2),
                in_=an[:],
            )
            nc.sync.dma_start(out=attnT[pair][:, qsl], in_=attn_d[qsl, csl], transpose=True)

        def emit_proj(qc, tt, ec):
            tsl = slice(128 * tt, 128 * (tt + 1))
            esl = slice(512 * ec, 512 * (ec + 1))
            ps = p_big.tile([128, 512], dt.float32, tag="psb", name="psb")
            for m in range(NM):
                nc.tensor.matmul(ps[:], attnT[m][:, tsl], wp_sb[:, m, esl],
                                 start=(m == 0), stop=(m == NM - 1))
            ysb = p_y.tile([128, 512], dt.float32, tag="ysb", name="ysb")
            nc.vector.tensor_copy(ysb[:], ps[:])
            nc.gpsimd.dma_start(out=y[tsl, esl], in_=ysb[:])

        # ================= build-time list scheduler =======================
        # Global S order honours staging epochs: epoch(w, kt) =
        # max(qc, kt//4); within an epoch, window-major so windows finish
        # (and release norm + projection work) as early as possible.
        s_order = sorted(
            [(w, kt) for w in range(NW) for kt in range(NTT)],
            key=lambda u: (max(u[0] // NM, u[1] // 4), u[0], u[1]),
        )

        def s_deps(w, kt):
            qc, pair = windows[w]
            b, half = kt // 4, (kt % 4) // 2
            d = []
            if qc == 0:
                d += [("pq", 0, pair, 0), ("pq", 0, pair, 1)]
            else:
                d += [("pq", qc, pair, 0)]
            if b == 0:
                d += [("pk", 0, pair, h) for h in range(half + 1)]
            else:
                d += [("pk", b, pair, 0)]
            return d

        def av_deps(w, kt):
            return [("pv", kt // 4, kt % 4)]

        t_pe = [0.0]
        t_act = [0.0]
        t_dve = [0.0]
        av_free = [0.0]
        exp_end = {}

        plan_order = list(stagers.keys())

        def evict_target(key):
            cls = 256 if key[2] is not None else 512
            done_cls = [k for k in loads_emitted
                        if (256 if k[2] is not None else 512) == cls]
            if len(done_cls) < 4:
                return None
            return done_cls[-4]

        def maybe_stage():
            # pre-issue the next planned XPOSE once the readers of the slot
            # it would evict have all been emitted (keeps the DMA transposes
            # ahead of their consumer chains)
            for key in plan_order:
                if stagers[key]["done"]:
                    continue
                tgt = evict_target(key)
                if tgt is not None and any(
                    not chains[cid]["done"] for cid in readers.get(tgt, [])
                ):
                    return
                stagers[key]["done"] = True
                loads_emitted.append(key)
                do_load(key)
                return

        def urgent_chains():
            for key in plan_order:
                if not stagers[key]["done"]:
                    tgt = evict_target(key)
                    if tgt is None:
                        return []
                    return [c for c in readers.get(tgt, [])
                            if not chains[c]["done"]]
            return []

        def run_chain(cid):
            c = chains[cid]
            if c["done"]:
                return
            c["done"] = True
            ensure_staged(c["key"])
            t_pe[0] = max(t_pe[0], c["ready"]) + c["dur"]
            t_dve[0] = max(t_dve[0], t_pe[0] + SEM) + DVE_COPY
            c["fn"]()

        def s_ready_est(idx, w, kt):
            r = 0.0
            for cid in s_deps(w, kt):
                c = chains[cid]
                r = max(r, (c["ready"] + c["dur"] if not c["done"] else 0.0))
            if idx >= 2:
                r = max(r, exp_end[idx - 2] + SEM)
            return r

        proj_q = []          # (ready est, qc, tt, ec)
        xb_est = {}          # (qc, pair) -> est attnT ready
        s_idx = [0]
        av_idx = [0]

        def do_S():
            w, kt = s_order[s_idx[0]]
            for cid in s_deps(w, kt):
                run_chain(cid)
            t_pe[0] = max(t_pe[0], s_ready_est(s_idx[0], w, kt)) + 2 * 512 * MM
            emit_S(w, kt)
            t_act[0] = max(t_act[0], t_pe[0] + SEM) + EXP_NS
            exp_end[s_idx[0]] = t_act[0]
            s_idx[0] += 1

        def do_AV():
            w, kt = s_order[av_idx[0]]
            for cid in av_deps(w, kt):
                run_chain(cid)
            lo = max(t_pe[0], exp_end[av_idx[0]] + SEM)
            if kt == 0:
                lo = max(lo, av_free[0])
            t_pe[0] = lo + 8 * 65 * MM
            emit_AV(w, kt)
            av_idx[0] += 1
            if kt == NTT - 1:
                qc, pair = windows[w]
                emit_norm(w)
                t_dve[0] = max(t_dve[0], t_pe[0] + SEM) + 900
                av_free[0] = t_dve[0] + SEM
                t_dve[0] += 1400
                xb_est[(qc, pair)] = t_dve[0] + 6000
                if pair == NM - 1:
                    rdy = max(xb_est[(qc, p)] for p in range(NM))
                    for tt in range(4 * qc, 4 * qc + 4):
                        for ec in range(2):
                            proj_q.append([rdy, qc, tt, ec])

        PROJ_RESERVE = 8

        def next_filler():
            # eviction-blocking chains first (they gate the next XPOSE), then
            # any ready chain, then ready projection tiles (minus a reserve
            # kept to cover the tail wait on the last window's transpose)
            for cid in urgent_chains():
                if chains[cid]["ready"] <= t_pe[0] + 400:
                    return ("chain", cid)
            for cid in chain_order:
                c = chains[cid]
                if not c["done"] and c["ready"] <= t_pe[0] + 100:
                    return ("chain", cid)
            navail = len(proj_q) - (PROJ_RESERVE if s_idx[0] < NSU else 0)
            for it in proj_q[:max(0, navail)]:
                if it[0] <= t_pe[0] + 100:
                    return ("proj", it)
            return None

        def do_filler(f):
            kind, it = f
            if kind == "chain":
                run_chain(it)
            else:
                proj_q.remove(it)
                t_pe[0] = max(t_pe[0], it[0]) + 4 * 512 * MM
                emit_proj(it[1], it[2], it[3])
                t_dve[0] = max(t_dve[0], t_pe[0] + SEM) + DVE_COPY

        NSU = len(s_order)
        while s_idx[0] < NSU or av_idx[0] < NSU or proj_q or not all(
            c["done"] for c in chains.values()
        ):
            maybe_stage()
            can_S = s_idx[0] < NSU
            # AV trails S by >=2 units so the exp result is comfortably ready
            can_AV = av_idx[0] < min(s_idx[0] - (2 if can_S else 0), NSU)
            s_stall = (
                max(0.0, s_ready_est(s_idx[0], *s_order[s_idx[0]]) - t_pe[0])
                if can_S else float("inf")
            )
            av_stall = (
                max(0.0, exp_end[av_idx[0]] + SEM - t_pe[0]) if can_AV else float("inf")
            )
            if can_AV and s_idx[0] - av_idx[0] >= ES_BUFS - 4:
                do_AV()
                continue
            if can_S and s_stall <= 30:
                do_S()
                continue
            if can_AV and av_stall <= 30:
                do_AV()
                continue
            f = next_filler()
            if f is not None:
                do_filler(f)
                continue
            if can_AV and av_stall <= s_stall:
                do_AV()
            elif can_S:
                do_S()
            elif can_AV:
                do_AV()
            elif proj_q:
                it = min(proj_q, key=lambda x: x[0])
                do_filler(("proj", it))
            else:
                # stragglers: emit remaining chains by readiness
                rem = [cid for cid in chain_order if not chains[cid]["done"]]
                run_chain(min(rem, key=lambda c: chains[c]["ready"]))

    nc.compile()
    return nc


def _get_nc():
    if "nc" not in _CACHE:
        _CACHE["nc"] = _build_nc()
    return _CACHE["nc"]


def core_input_map(k, q, v, w_key, w_query, w_value, w_proj, core):
    b, g = core // 2, core % 2
    sl = slice(g * HGD, (g + 1) * HGD)
    f32 = np.float32
    return {
        "xq": np.ascontiguousarray(q[b], dtype=f32),
        "xk": np.ascontiguousarray(k[b], dtype=f32),
        "xv": np.ascontiguousarray(v[b], dtype=f32),
        "wq": np.ascontiguousarray(w_query[:, sl], dtype=f32),
        "wk": np.ascontiguousarray(w_key[:, sl], dtype=f32),
        "wv": np.ascontiguousarray(w_value[:, sl], dtype=f32),
        "wp": np.ascontiguousarray(w_proj[sl, :], dtype=f32),
    }


def kernel(k, q, v, w_key, w_query, w_value, w_proj):
    from concourse.bass_utils import run_bass_kernel_spmd

    nc = _get_nc()
    in_maps = [
        core_input_map(k, q, v, w_key, w_query, w_value, w_proj, c) for c in range(8)
    ]
    res = run_bass_kernel_spmd(nc, in_maps, list(range(8))).results
    out = np.empty((B, T, EMB), np.float32)
    for b in range(B):
        out[b] = res[2 * b]["y"] + res[2 * b + 1]["y"]
    return out
